# revision 29
# baseline (speedup 1.0000x reference)
"""Grouped SwiGLU MoE MLP (16 experts) on 8 NeuronCores, expert-parallel.

Reference computation, per expert e over its contiguous token slice xi:
    out = (silu(xi @ w_gate[e].T) * (xi @ w_up[e].T)) @ w_down[e].T

Sharding: expert-parallel. Core c owns experts {2c, 2c+1}; the host hands it
the matching contiguous 2048-token slice of x (tokens are pre-sorted by
expert), so no device-side collectives are needed.

All matmul operands are bf16 (host-cast): the PE streaming rate matches
fp32r but FWL halves LDWEIGHTS time and input DMA bytes halve. PSUM
accumulation stays fp32 and the output is fp32 (~4e-3 rel err, inside the
2e-2 gate).

Layout: pre-packed on the host so every DMA is a contiguous block:
  xq  [EPC, TH, P, HT, NT] xq[e,th,p,a,n] = x[e*TPE+th*NT+n, a*P+p]
  wgq [EPC, FT, P, HT*P]  wgq[e,f,p,a*P+j] = w_gate[e, f*P+j, a*P+p]
  wuq  same as wgq for w_up
  wdq [EPC, HG, P, FT*P]  wdq[e,g,p,f*P+j] = w_down[e, g*P+j, f*P+p]
  outq[EPC, HG, P, TPE]   outq[e,g,p,t] = out[e*TPE+t, g*P+p]

Schedule (from trace analysis of earlier revisions):
- The framework preamble ends ~6.5us and the first DMA packet lands ~8us;
  NWARM dummy matmuls on a zeroed tile hold HAM at K=8/8 through that
  window so real work starts at 2.4GHz.
- Chains are t-half-outer so the first f-tile only needs the th=0 half of
  x (1MB instead of 2MB) before the PE can run 32 back-to-back matmuls,
  and so each t-half's PSUM pair drains while the other half's chains run.
- The Scalar queue runs ONLY the silu ACTIVATEs: a dma_start whose
  pool-pacing semaphore isn't yet satisfied parks its whole queue, and
  parking the silu queue delays PSUM consumption and costs the PE one
  matmul slot per group. All steady-state DMA goes on the Sync ring,
  ordered so no paced load ever sits ahead of a store it would block.
- Expert 0's first-tile weights + x are split across both rings for
  startup bandwidth (the Scalar queue is empty until the first silu).
"""

import numpy as np
import ml_dtypes

import concourse.bass as bass
import concourse.bacc as bacc
import concourse.mybir as mybir
from concourse import tile
from concourse.bass_utils import run_bass_kernel_spmd

E, T, H, F = 16, 16384, 1024, 2048
NCORES = 8
EPC = E // NCORES          # experts per core
TPE = T // E               # tokens per expert (uniform fast path)
P = 128                    # SBUF partitions
HT = H // P                # 8 h-tiles (contraction tiles for gate/up)
FT = F // P                # 16 f-tiles
HGS = H // P               # 8 output h-groups for down proj
NT = 512                   # matmul moving free dim (PSUM bank = 512 fp32)
TH = TPE // NT             # 2 t-halves
NWARM = 24                 # dummy matmuls to pre-warm the PE clock
# Chains accumulate h-tiles starting at 4: the scalar ring's x half
# (h-tiles 4-7) lands ~1.5us before the sync ring's (0-3) at startup.
HT_ORDER = [4, 5, 6, 7, 0, 1, 2, 3]

BF16 = mybir.dt.bfloat16
F32 = mybir.dt.float32
BF16_NP = ml_dtypes.bfloat16

_CACHE = {}

# Set by run for test harness introspection (exec_time_ns, profile).
LAST_RESULTS = None
TRACE = False
TRACE_KW = {}


def _build_nc():
    nc = bacc.Bacc()
    xq = nc.dram_tensor("xq", [EPC, TH, P, HT, NT], BF16, kind="ExternalInput")
    wgq = nc.dram_tensor("wgq", [EPC, FT, P, HT * P], BF16, kind="ExternalInput")
    wuq = nc.dram_tensor("wuq", [EPC, FT, P, HT * P], BF16, kind="ExternalInput")
    wdq = nc.dram_tensor("wdq", [EPC, HGS, P, FT * P], BF16, kind="ExternalInput")
    outq = nc.dram_tensor("outq", [EPC, HGS, P, TPE], F32, kind="ExternalOutput")

    with tile.TileContext(nc) as tc:
        with (
            tc.tile_pool(name="xp", bufs=4) as xp,
            tc.tile_pool(name="wgp", bufs=5) as wgp,
            tc.tile_pool(name="wup", bufs=5) as wup,
            tc.tile_pool(name="wdp", bufs=8) as wdp,
            tc.tile_pool(name="hid", bufs=FT + 1) as hidp,
            tc.tile_pool(name="tmp", bufs=5) as tmpp,
            tc.tile_pool(name="osb", bufs=4) as osbp,
            tc.tile_pool(name="ps", bufs=8, space=bass.MemorySpace.PSUM) as psp,
        ):
            # PE warm-up (see module docstring). The dummy stream must end
            # no earlier than the startup DMA lands (13.5-17.3us observed,
            # run-to-run) or the PE idles, HAM re-throttles, and the first
            # real chains run at half clock (~2.5us penalty); 24 N=512
            # dummies bridge from ~7.7us to ~17.1us. The memset runs on
            # the DVE, which is idle until the first mul.
            warm = tmpp.tile([P, NT], BF16, tag="warm", name="warm")
            nc.vector.memset(warm[:], 0.0)
            wps = psp.tile([P, NT], F32, tag="ps", name="warm_ps")
            for _ in range(NWARM):
                nc.tensor.matmul(wps[:], warm[:, 0:P], warm[:],
                                 start=True, stop=True)

            xts = {}
            wgts = {}
            wuts = {}

            def prefetch_head(el):
                """First f-tile weights + x. For expert 0 this is the
                startup-critical burst and is split across both rings; the
                Scalar queue is otherwise empty until the first silu. The
                t-halves of x are SEPARATE tiles so the th=0 chains' matmul
                dependency doesn't round up to the th=1 DMAs."""
                for th in range(TH):
                    xtt = xp.tile([P, HT, NT], BF16, tag="xt",
                                  name=f"x{el}_{th}")
                    xts[(el, th)] = xtt
                wgt = wgp.tile([P, HT * P], BF16, tag="wg", name=f"wg{el}_0")
                nc.sync.dma_start(wgt[:], wgq[el, 0])
                wgts[(el, 0)] = wgt
                # x's th=0 half before wu: the up-chain consumes wu only
                # 8 matmuls after the gate chain starts, but every chain
                # needs x.
                nc.scalar.dma_start(xts[(el, 0)][:, 4:8, :], xq[el, 0][:, 4:8, :])
                wut = wup.tile([P, HT * P], BF16, tag="wu", name=f"wu{el}_0")
                nc.scalar.dma_start(wut[:], wuq[el, 0])
                wuts[(el, 0)] = wut
                nc.sync.dma_start(xts[(el, 0)][:, 0:4, :], xq[el, 0][:, 0:4, :])
                nc.sync.dma_start(xts[(el, 1)][:, 0:4, :], xq[el, 1][:, 0:4, :])
                nc.scalar.dma_start(xts[(el, 1)][:, 4:8, :], xq[el, 1][:, 4:8, :])

            prefetch_head(0)
            for el in range(EPC):
                for ft in range(1, FT):
                    wgt = wgp.tile([P, HT * P], BF16, tag="wg", name=f"wg{el}_{ft}")
                    nc.sync.dma_start(wgt[:], wgq[el, ft])
                    wgts[(el, ft)] = wgt
                    wut = wup.tile([P, HT * P], BF16, tag="wu", name=f"wu{el}_{ft}")
                    nc.sync.dma_start(wut[:], wuq[el, ft])
                    wuts[(el, ft)] = wut
                wdts = {}
                for hg in range(HGS):
                    wdt = wdp.tile([P, FT * P], BF16, tag="wd", name=f"wd{el}_{hg}")
                    nc.sync.dma_start(wdt[:], wdq[el, hg])
                    wdts[hg] = wdt

                # Gate/up, t-half-outer: for each f-tile, run the gate and
                # up chains for th=0 (16 MMs), whose silu+mul drain while
                # the th=1 chains (16 MMs) run.
                hids = []
                for ft in range(FT):
                    wgt, wut = wgts.pop((el, ft)), wuts.pop((el, ft))
                    hid = hidp.tile([P, TPE], BF16, tag="hid", name=f"hid{el}_{ft}")
                    for th in range(TH):
                        tsl = slice(th * NT, (th + 1) * NT)
                        xtt = xts[(el, th)]
                        g_ps = psp.tile([P, NT], F32, tag="ps", name=f"g{el}_{ft}_{th}")
                        u_ps = psp.tile([P, NT], F32, tag="ps", name=f"u{el}_{ft}_{th}")
                        for i, ht in enumerate(HT_ORDER):
                            nc.tensor.matmul(
                                g_ps[:], wgt[:, ht * P:(ht + 1) * P],
                                xtt[:, ht, :],
                                start=(i == 0), stop=(i == HT - 1),
                            )
                        for i, ht in enumerate(HT_ORDER):
                            nc.tensor.matmul(
                                u_ps[:], wut[:, ht * P:(ht + 1) * P],
                                xtt[:, ht, :],
                                start=(i == 0), stop=(i == HT - 1),
                            )
                        tmp = tmpp.tile([P, NT], BF16, tag="tmp")
                        nc.scalar.activation(
                            tmp[:], g_ps[:],
                            mybir.ActivationFunctionType.Silu,
                        )
                        nc.vector.tensor_mul(hid[:, tsl], tmp[:], u_ps[:])
                    hids.append(hid)

                # Next expert's head prefetch goes out before this expert's
                # output stores so its x/weights are resident at the
                # expert boundary.
                if el + 1 < EPC:
                    prefetch_head(el + 1)

                # Down projection, t-half-outer: th=0's 16-MM chain
                # completes before th=1's starts, so its copy+store overlap
                # the th=1 chain and the post-loop tail is one store deep.
                for hg in range(HGS):
                    wdt = wdts[hg]
                    for th in range(TH):
                        tsl = slice(th * NT, (th + 1) * NT)
                        if el == EPC - 1 and hg == HGS - 1 and th == TH - 1:
                            # Final t-half: two N=256 chains into SEPARATE
                            # PSUM banks (a shared bank would make the first
                            # chain's copy collide with the second chain's
                            # writes), so the first quarter's store overlaps
                            # the second chain and the post-loop tail is one
                            # 64KB-store deep per queue.
                            # Sub-chain widths 256/128/128: each earlier
                            # piece's copy+store overlaps the later chains,
                            # so only one 64KB store per queue remains after
                            # the last matmul (store latency, not bandwidth,
                            # dominates the tail).
                            widths = [NT // 2, NT // 4, NT // 4]
                            offs = [0, NT // 2, 3 * NT // 4]
                            d_qs = [psp.tile([P, NT], F32, tag="ps",
                                             name=f"d{el}_{hg}_{th}_{q}")
                                    for q in range(3)]
                            osb = osbp.tile([P, NT], F32, tag="osb")
                            for q in range(3):
                                w, o = widths[q], offs[q]
                                for ft in range(FT):
                                    nc.tensor.matmul(
                                        d_qs[q][:, 0:w],
                                        wdt[:, ft * P:(ft + 1) * P],
                                        hids[ft][:, th * NT + o:
                                                  th * NT + o + w],
                                        start=(ft == 0), stop=(ft == FT - 1),
                                    )
                                osl = slice(o, o + w)
                                dst = outq[el, hg][:, th * NT + o:
                                                  th * NT + o + w]
                                if q < 2:
                                    nc.vector.tensor_copy(osb[:, osl],
                                                          d_qs[q][:, 0:w])
                                    eng = nc.scalar if q == 0 else nc.sync
                                    eng.dma_start(dst, osb[:, osl])
                                else:
                                    nc.scalar.activation(
                                        osb[:, osl], d_qs[q][:, 0:w],
                                        mybir.ActivationFunctionType.Copy,
                                    )
                                    nc.scalar.dma_start(dst, osb[:, osl])
                        else:
                            d_ps = psp.tile([P, NT], F32, tag="ps",
                                            name=f"d{el}_{hg}_{th}")
                            for ft in range(FT):
                                nc.tensor.matmul(
                                    d_ps[:], wdt[:, ft * P:(ft + 1) * P],
                                    hids[ft][:, tsl],
                                    start=(ft == 0), stop=(ft == FT - 1),
                                )
                            osb = osbp.tile([P, NT], F32, tag="osb")
                            nc.vector.tensor_copy(osb[:], d_ps[:])
                            nc.sync.dma_start(outq[el, hg][:, tsl], osb[:])
    return nc


def get_nc():
    if "nc" not in _CACHE:
        nc = _build_nc()
        nc.finalize()
        _CACHE["nc"] = nc
    return _CACHE["nc"]


def make_in_maps(x, w_gate, w_up, w_down):
    xb = x.astype(BF16_NP)
    wgb = w_gate.astype(BF16_NP)
    wub = w_up.astype(BF16_NP)
    wdb = w_down.astype(BF16_NP)
    in_maps = []
    for c in range(NCORES):
        e0 = c * EPC
        # xq[e,th,p,a,n] = x[e*TPE + th*NT + n, a*P+p]
        xs = xb[e0 * TPE:(e0 + EPC) * TPE].reshape(EPC, TH, NT, HT, P)
        xqc = np.ascontiguousarray(xs.transpose(0, 1, 4, 3, 2))
        # wgq[e,f,p,a*P+j] = w_gate[e, f*P+j, a*P+p]
        wg = wgb[e0:e0 + EPC].reshape(EPC, FT, P, HT, P)
        wgc = np.ascontiguousarray(wg.transpose(0, 1, 4, 3, 2)).reshape(
            EPC, FT, P, HT * P)
        wu = wub[e0:e0 + EPC].reshape(EPC, FT, P, HT, P)
        wuc = np.ascontiguousarray(wu.transpose(0, 1, 4, 3, 2)).reshape(
            EPC, FT, P, HT * P)
        # wdq[e,g,p,f*P+j] = w_down[e, g*P+j, f*P+p]
        wd = wdb[e0:e0 + EPC].reshape(EPC, HGS, P, FT, P)
        wdc = np.ascontiguousarray(wd.transpose(0, 1, 4, 3, 2)).reshape(
            EPC, HGS, P, FT * P)
        in_maps.append({"xq": xqc, "wgq": wgc, "wuq": wuc, "wdq": wdc})
    return in_maps


def _numpy_fallback(x, w_gate, w_up, w_down, counts):
    out = np.empty((x.shape[0], w_down.shape[1]), np.float32)
    o = 0
    for e in range(len(counts)):
        n = int(counts[e])
        xi = x[o:o + n]
        gate = xi @ w_gate[e].T
        up = xi @ w_up[e].T
        hidden = (gate / (1.0 + np.exp(-gate))) * up
        out[o:o + n] = hidden @ w_down[e].T
        o += n
    return out


def kernel(x, w_gate, w_up, w_down, tokens_per_expert):
    global LAST_RESULTS
    x = np.asarray(x, dtype=np.float32)
    w_gate = np.asarray(w_gate, dtype=np.float32)
    w_up = np.asarray(w_up, dtype=np.float32)
    w_down = np.asarray(w_down, dtype=np.float32)
    counts = np.asarray(tokens_per_expert).astype(np.int64)

    if not (counts.shape == (E,) and np.all(counts == TPE)):
        # Non-uniform routing: the compiled program is shaped for the
        # uniform split the reference generator produces.
        return _numpy_fallback(x, w_gate, w_up, w_down, counts)

    nc = get_nc()
    res = run_bass_kernel_spmd(
        nc, make_in_maps(x, w_gate, w_up, w_down), list(range(NCORES)),
        trace=TRACE, **TRACE_KW,
    )
    LAST_RESULTS = res
    out = np.empty((T, H), np.float32)
    for c in range(NCORES):
        o = res.results[c]["outq"]  # [EPC, HGS, P, TPE]
        for el in range(EPC):
            t0 = (c * EPC + el) * TPE
            # out[t0+t, g*P+p] = o[el, g, p, t]
            out[t0:t0 + TPE] = o[el].transpose(2, 0, 1).reshape(TPE, H)
    return out
